# revision 24
# baseline (speedup 1.0000x reference)
"""Distributed causal multi-head attention for 8 TRN2 NeuronCores (v2, fp16).

Problem: B=4, S=2048, D=1024, H=16 heads of DH=64, fp32 in/out, causal mask.

Sharding: core c -> (batch b = c//2, head-group g = c%2 of 8 heads).

v2 changes vs baseline:
- fp16 datapath end-to-end (host pre-casts inputs; matmuls 1 cyc/row vs
  fp32r's 2; halved DMA + SBUF footprint). Verified numerics: ~5e-4 rel.
- attention software-pipelined with lag-3: scores(h,i)+exp emitted 3
  slots ahead of AV(h,i) so the PE never waits on the Scalar-engine exp
  (which is the #2 engine at ~150us total).
- diagonal causal mask via one elementwise multiply with a constant
  triangular tile (gpsimd) instead of affine_select.
- attT kept in SBUF (no DRAM round-trip before the output projection).
- softmax normalize: reciprocal of the denominator row directly (1,512),
  partition_broadcast, one multiply PSUM->SBUF(fp16).
"""

import numpy as np

import concourse.bass as bass
import concourse.mybir as mybir
import concourse.tile as tile
from concourse import bacc

B, S, D, H = 4, 2048, 1024, 16
DH = 64
NG = 2              # head groups (cores per batch)
DG = D // NG        # 512 head dims per core
HL = H // NG        # 8 heads per core
PB = 128            # partition block
CH = 512            # free-dim chunk (one fp32 PSUM bank)
NCH = S // CH       # 4 chunks
NKT = S // PB       # 16 key tiles
NDT = D // PB       # 8 contraction tiles for projections
NJT = DG // PB      # 4 head-dim tiles per core
HS = S // 2         # 1024, half of seq
F32 = mybir.dt.float32
F16 = mybir.dt.float16
SCALE = 1.0 / 8.0   # 1/sqrt(DH)
LAG = 3             # attention pipeline depth in (head, key-tile) slots


def _emit(nc, xq, xkv, wq, wk, wv, wo, pb, cmask, outT):
    with tile.TileContext(nc) as tc:
        with (
            tc.tile_pool(name="pers", bufs=1) as pers,
            tc.tile_pool(name="xp", bufs=1) as xp,
            tc.tile_pool(name="wp", bufs=1) as wp,
            tc.tile_pool(name="wo", bufs=1) as wop,
            tc.tile_pool(name="qt", bufs=1) as qtp,
            tc.tile_pool(name="kt", bufs=1) as ktp,
            tc.tile_pool(name="vt", bufs=1) as vtp,
            tc.tile_pool(name="at", bufs=1) as attp,
            tc.tile_pool(name="ex", bufs=LAG + 2) as exp_pool,
            tc.tile_pool(name="rc", bufs=2) as rcp,
            tc.tile_pool(name="ost", bufs=3) as ostp,
            tc.tile_pool(name="ps", bufs=1, space="PSUM") as ps,
        ):
            # ---------------- persistent small tiles ----------------
            pbias_sb = pers.tile([PB, NKT], F32, tag="pbias", name="pbias_sb")
            nc.sync.dma_start(out=pbias_sb[:], in_=pb[:].rearrange("(i p) -> p i", p=PB))


            # ---------------- long-lived activation tiles ----------------
            qt = [qtp.tile([PB, S], F16, tag=f"qt{j}", name=f"qt{j}") for j in range(NJT)]
            kt = [ktp.tile([PB, S], F16, tag=f"kt{j}", name=f"kt{j}") for j in range(NJT)]
            # V with one extra "ones" column per head: (128, 8*65)
            vt = [vtp.tile([PB, HL * (DH + 1)], F16, tag=f"vt{i}", name=f"vt{i}") for i in range(NKT)]
            ones8 = pers.tile([PB, HL], F16, tag="ones8", name="ones8")
            nc.gpsimd.memset(ones8[:], 1.0)
            for i in range(NKT):
                ones_view = vt[i][:].rearrange("p (h c) -> p h c", c=DH + 1)[:, :, DH]
                nc.vector.tensor_copy(ones_view, ones8[:])
            # attention output, transposed layout: att_sb[j][r, q] with
            # r = head-dim row within block j (heads 2j, 2j+1)
            att_sb = [attp.tile([PB, S], F16, tag=f"at{j}", name=f"at{j}") for j in range(NJT)]

            # output-projection weights up front
            wol = []
            for j in range(NJT):
                t = wop.tile([PB, D], F16, tag=f"wo{j}", name=f"wo{j}")
                nc.sync.dma_start(out=t[:], in_=wo[j * PB:(j + 1) * PB, :])
                wol.append(t)

            # PSUM: four (128, 1024) two-bank pair tiles, tags A..D
            def pair_tile(tag):
                return ps.tile([PB, 2 * CH], F32, tag=tag, name=f"ps{tag}")

            def halves(t):
                return [t[:, 0:CH], t[:, CH:2 * CH]]

            def load_w(dram_w, d):
                t = wp.tile([PB, DG], F16, tag=f"w{d}", name=f"w{d}")
                nc.sync.dma_start(out=t[:], in_=dram_w[d * PB:(d + 1) * PB, :])
                return t

            def load_xh(dram_x, d, half):
                t = xp.tile([PB, HS], F16, tag=f"b{half * NDT + d}", name=f"xh{d}")
                nc.sync.dma_start(
                    out=t[:], in_=dram_x[d * PB:(d + 1) * PB,
                                         half * HS:(half + 1) * HS])
                return t

            # ---------------- Q projection (8 groups on A..D) ----------------
            wq_tiles = [load_w(wq, d) for d in range(NDT)]
            xq_halves = [[load_xh(xq, d, hf) for d in range(NDT)] for hf in range(2)]
            for half in range(2):
                xh = xq_halves[half]
                accs = []
                for tag in "ABCD":
                    accs += halves(pair_tile(tag))
                wts = wq_tiles
                for d in range(NDT):
                    for j in range(NJT):
                        for ci in range(2):
                            nc.tensor.matmul(
                                accs[j * 2 + ci],
                                wts[d][:, j * PB:(j + 1) * PB],
                                xh[d][:, ci * CH:(ci + 1) * CH],
                                start=(d == 0), stop=(d == NDT - 1),
                            )
                for j in range(NJT):
                    for ci in range(2):
                        c = half * 2 + ci
                        nc.vector.tensor_copy(
                            qt[j][:, c * CH:(c + 1) * CH], accs[j * 2 + ci])

            # ------------- K/V projections (4-group passes on A/B) -------------
            for half in range(2):
                wts = [load_w(wk, d) for d in range(NDT)]
                xh = [load_xh(xkv, d, half) for d in range(NDT)]
                for jp in range(2):
                    accs = halves(pair_tile("A")) + halves(pair_tile("B"))
                    for d in range(NDT):
                        for jj in range(2):
                            j = jp * 2 + jj
                            for ci in range(2):
                                nc.tensor.matmul(
                                    accs[jj * 2 + ci],
                                    wts[d][:, j * PB:(j + 1) * PB],
                                    xh[d][:, ci * CH:(ci + 1) * CH],
                                    start=(d == 0), stop=(d == NDT - 1),
                                )
                    for jj in range(2):
                        j = jp * 2 + jj
                        for ci in range(2):
                            c = half * 2 + ci
                            nc.vector.tensor_copy(
                                kt[j][:, c * CH:(c + 1) * CH], accs[jj * 2 + ci])
                wvs = [load_w(wv, d) for d in range(NDT)]
                for sp in range(2):
                    accs = halves(pair_tile("A")) + halves(pair_tile("B"))
                    for d in range(NDT):
                        for s4 in range(4):
                            si = sp * 4 + s4
                            nc.tensor.matmul(
                                accs[s4],
                                xh[d][:, si * PB:(si + 1) * PB],
                                wvs[d][:],
                                start=(d == 0), stop=(d == NDT - 1),
                            )
                    for s4 in range(4):
                        i = half * 8 + sp * 4 + s4
                        src = accs[s4].rearrange("p (h c) -> p h c", c=DH)
                        dst = vt[i][:].rearrange("p (h c) -> p h c", c=DH + 1)[:, :, 0:DH]
                        nc.vector.tensor_copy(dst, src)

            # ---------------- attention, lag-LAG pipelined ----------------
            # scores/exp on C/D pair tiles; AV accumulators: chunk c ->
            # half (c%2) of pair (A if c<2 else B), per head.
            state = {"st_cnt": 0, "opair": None}

            def emit_scores(h, i):
                jq = h // 2
                rowo = (h % 2) * DH
                c0 = i // 4
                ex_t = exp_pool.tile([PB, S], F16, tag="ex", name=f"ex_{h}_{i}")
                for hh in range(c0 // 2, 2):
                    st_t = pair_tile("CD"[state["st_cnt"] % 2])
                    state["st_cnt"] += 1
                    lo_c = max(c0, hh * 2)
                    for c in range(lo_c, hh * 2 + 2):
                        q_lo = max(c * CH, i * PB)
                        nc.tensor.matmul(
                            st_t[:, q_lo - hh * 2 * CH:(c - hh * 2 + 1) * CH],
                            kt[jq][rowo:rowo + DH, i * PB:(i + 1) * PB],
                            qt[jq][rowo:rowo + DH, q_lo:(c + 1) * CH],
                            start=True, stop=True,
                        )
                    s0 = max(lo_c * CH, i * PB)
                    span = (hh + 1) * 2 * CH - s0
                    nc.scalar.activation(
                        ex_t[:, s0:s0 + span],
                        st_t[:, s0 - hh * 2 * CH:s0 - hh * 2 * CH + span],
                        mybir.ActivationFunctionType.Exp,
                        bias=pbias_sb[:, i:i + 1], scale=SCALE,
                    )
                # zero q < k inside the 128-wide diagonal block (gpsimd:
                # same firmware lib as partition_broadcast -- no lib thrash)
                nc.gpsimd.affine_select(
                    out=ex_t[:, i * PB:(i + 1) * PB],
                    in_=ex_t[:, i * PB:(i + 1) * PB],
                    compare_op=mybir.AluOpType.is_ge, fill=0.0,
                    base=0, pattern=[[1, PB]],
                    channel_multiplier=-1,
                )
                return ex_t

            def oaug(h, c):
                return state["opair"][c // 2][:, (c % 2) * CH:(c % 2 + 1) * CH]

            def att_rows(h):
                return att_sb[h // 2][(h % 2) * DH:(h % 2) * DH + DH, :]

            def chunk_done(h, c, stg, dnh):
                # free the PSUM bank fast: O rows -> staging at partitions
                # 0..63, denominator row -> dnh at partition 64 (engines
                # cannot shift partitions; DMA cannot read PSUM)
                oa = oaug(h, c)
                nc.vector.tensor_copy(stg[:, c * CH:(c + 1) * CH], oa[0:DH, :])
                nc.vector.tensor_copy(dnh[DH:DH + 1, c * CH:(c + 1) * CH],
                                      oa[DH:DH + 1, :])

            def head_done(h, dnh):
                # reciprocal of all 2048 denominators via the partition-
                # transpose DMA trick (recip free-size 16, not 2048), then
                # broadcast 1/den across partitions. The normalize multiply
                # itself is deferred (emitted a head later) so this chain
                # never blocks the DVE queue.
                dnp = rcp.tile([PB, NKT], F32, tag="dnp", name="dnp_t")
                nc.sync.dma_start(out=dnp[:], in_=dnh[DH:DH + 1, :])
                rcs = rcp.tile([PB, NKT], F32, tag="rcs", name="rcs_t")
                with nc.allow_low_precision(reason="softmax recip"):
                    nc.vector.reciprocal(rcs[:], dnp[:])
                rc2 = rcp.tile([1, S], F32, tag="rc2", name="rc2_t")
                nc.sync.dma_start(out=rc2[:], in_=rcs[:])
                bc = rcp.tile([DH, S], F32, tag="bc", name="bc_t")
                nc.gpsimd.partition_broadcast(bc[:], rc2[0:1, :])
                return bc

            def head_mult(h, stg, bc):
                nc.vector.tensor_tensor(att_rows(h), stg[:], bc[:],
                                        mybir.AluOpType.mult)

            def emit_av(h, i, ex_t):
                if i == 0:
                    state["opair"] = [pair_tile("A"), pair_tile("B")]
                    state["dnh"] = rcp.tile([DH + 1, S], F32, tag="dnh", name="dnh_t")
                    state["stg"] = rcp.tile([DH, S], F16, tag="stg", name="stg_t")
                    # deferred normalize of the PREVIOUS head: its bc has
                    # had a full head of slack to land
                    if state.get("pending") is not None:
                        head_mult(*state["pending"])
                        state["pending"] = None
                c0 = i // 4
                for c in range(c0, NCH):
                    if c == c0:
                        off = i * PB - c0 * CH
                        out_ap = oaug(h, c)[:, off:CH]
                        rhs = ex_t[:, i * PB:(c0 + 1) * CH]
                    else:
                        out_ap = oaug(h, c)
                        rhs = ex_t[:, c * CH:(c + 1) * CH]
                    nc.tensor.matmul(
                        out_ap[0:DH + 1, :],
                        vt[i][:, h * (DH + 1):(h + 1) * (DH + 1)],
                        rhs,
                        start=(i == 0), stop=(i == 4 * c + 3),
                    )
                    if i == 4 * c + 3:
                        chunk_done(h, c, state["stg"], state["dnh"])
                if i == NKT - 1:
                    bc = head_done(h, state["dnh"])
                    state["pending"] = (h, state["stg"], bc)

            seq = [(h, i) for h in range(HL) for i in range(NKT)]
            ex_map = {}
            for idx, (h, i) in enumerate(seq):
                ex_map[(h, i)] = emit_scores(h, i)
                if idx >= LAG:
                    h2, i2 = seq[idx - LAG]
                    emit_av(h2, i2, ex_map.pop((h2, i2)))
            for idx in range(len(seq) - LAG, len(seq)):
                h2, i2 = seq[idx]
                emit_av(h2, i2, ex_map.pop((h2, i2)))
            head_mult(*state["pending"])

            # ---------------- output projection ----------------
            for m in range(D // PB):
                for c in range(NCH):
                    acc = pair_tile("ABCD"[c % 4])[:, 0:CH]
                    for j in range(NJT):
                        nc.tensor.matmul(
                            acc,
                            wol[j][:, m * PB:(m + 1) * PB],
                            att_sb[j][:, c * CH:(c + 1) * CH],
                            start=(j == 0), stop=(j == NJT - 1),
                        )
                    ost = ostp.tile([PB, CH], F16, tag="ost", name="ost")
                    nc.vector.tensor_copy(ost[:], acc)
                    nc.sync.dma_start(
                        out=outT[m * PB:(m + 1) * PB, c * CH:(c + 1) * CH],
                        in_=ost[:])


def build_module():
    nc = bacc.Bacc()
    xq = nc.declare_dram_parameter("xqT", [D, S], F16, isOutput=False)
    xkv = nc.declare_dram_parameter("xkvT", [D, S], F16, isOutput=False)
    wq = nc.declare_dram_parameter("wqT", [D, DG], F16, isOutput=False)
    wk = nc.declare_dram_parameter("wkT", [D, DG], F16, isOutput=False)
    wv = nc.declare_dram_parameter("wvT", [D, DG], F16, isOutput=False)
    wo = nc.declare_dram_parameter("woT", [DG, D], F16, isOutput=False)
    pb = nc.declare_dram_parameter("pbias", [S], F32, isOutput=False)
    cm = nc.declare_dram_parameter("cmask", [2 * PB, PB], F16, isOutput=False)
    outT = nc.declare_dram_parameter("outT", [D, S], F16, isOutput=True)
    _emit(nc, xq, xkv, wq, wk, wv, wo, pb, cm, outT)
    nc.finalize()
    return nc


_NC = None


def _get_nc():
    global _NC
    if _NC is None:
        _NC = build_module()
    return _NC


def make_in_maps(q_raw, kv_raw, padding_mask, Wq, Wk, Wv, Wo):
    q_raw = np.asarray(q_raw, np.float32)
    kv_raw = np.asarray(kv_raw, np.float32)
    qT = np.ascontiguousarray(q_raw.transpose(0, 2, 1)).astype(np.float16)
    kvT = np.ascontiguousarray(kv_raw.transpose(0, 2, 1)).astype(np.float16)
    pbias = np.where(np.asarray(padding_mask) == 0, -1e9, 0.0).astype(np.float32)
    # rows 0..127: mneg = strict-upper -1000 (lhsT of the causal mask add,
    # so that (mneg.T)[k, q] = -1000 where k > q); rows 128..255: identity
    mneg = np.triu(np.full((PB, PB), -1000.0, np.float16), k=1)
    cmask = np.concatenate([mneg, np.eye(PB, dtype=np.float16)], axis=0)
    Wq, Wk, Wv, Wo = (np.asarray(w, np.float32) for w in (Wq, Wk, Wv, Wo))
    wqT = [np.ascontiguousarray(Wq[g * DG:(g + 1) * DG, :].T).astype(np.float16) for g in range(NG)]
    wkT = [np.ascontiguousarray(Wk[g * DG:(g + 1) * DG, :].T).astype(np.float16) for g in range(NG)]
    wvT = [np.ascontiguousarray(Wv[g * DG:(g + 1) * DG, :].T).astype(np.float16) for g in range(NG)]
    woT = [np.ascontiguousarray(Wo[:, g * DG:(g + 1) * DG].T).astype(np.float16) for g in range(NG)]
    in_maps = []
    for c in range(NG * B):
        b, g = divmod(c, NG)
        in_maps.append({
            "xqT": qT[b], "xkvT": kvT[b],
            "wqT": wqT[g], "wkT": wkT[g], "wvT": wvT[g], "woT": woT[g],
            "pbias": pbias[b], "cmask": cmask,
        })
    return in_maps


def kernel(q_raw, kv_raw, padding_mask, Wq, Wk, Wv, Wo):
    from concourse.bass_utils import run_bass_kernel_spmd

    nc = _get_nc()
    in_maps = make_in_maps(q_raw, kv_raw, padding_mask, Wq, Wk, Wv, Wo)
    res = run_bass_kernel_spmd(nc, in_maps, core_ids=list(range(NG * B)))
    out = np.empty((B, S, D), np.float32)
    for b in range(B):
        out[b] = (res.results[NG * b]["outT"].astype(np.float32)
                  + res.results[NG * b + 1]["outT"].astype(np.float32)).T
    return out


# revision 29
# speedup vs baseline: 1.0763x; 1.0763x over previous
"""Distributed causal multi-head attention for 8 TRN2 NeuronCores (v2, fp16).

Problem: B=4, S=2048, D=1024, H=16 heads of DH=64, fp32 in/out, causal mask.

Sharding: core c -> (batch b = c//2, head-group g = c%2 of 8 heads).

v2 changes vs baseline:
- fp16 datapath end-to-end (host pre-casts inputs; matmuls 1 cyc/row vs
  fp32r's 2; halved DMA + SBUF footprint). Verified numerics: ~5e-4 rel.
- attention software-pipelined with lag-3: scores(h,i)+exp emitted 3
  slots ahead of AV(h,i) so the PE never waits on the Scalar-engine exp
  (which is the #2 engine at ~150us total).
- diagonal causal mask via one elementwise multiply with a constant
  triangular tile (gpsimd) instead of affine_select.
- attT kept in SBUF (no DRAM round-trip before the output projection).
- softmax normalize: reciprocal of the denominator row directly (1,512),
  partition_broadcast, one multiply PSUM->SBUF(fp16).
"""

import numpy as np

import concourse.bass as bass
import concourse.mybir as mybir
import concourse.tile as tile
from concourse import bacc

B, S, D, H = 4, 2048, 1024, 16
DH = 64
NG = 2              # head groups (cores per batch)
DG = D // NG        # 512 head dims per core
HL = H // NG        # 8 heads per core
PB = 128            # partition block
CH = 512            # free-dim chunk (one fp32 PSUM bank)
NCH = S // CH       # 4 chunks
NKT = S // PB       # 16 key tiles
NDT = D // PB       # 8 contraction tiles for projections
NJT = DG // PB      # 4 head-dim tiles per core
HS = S // 2         # 1024, half of seq
F32 = mybir.dt.float32
F16 = mybir.dt.float16
SCALE = 1.0 / 8.0   # 1/sqrt(DH)
LAG = 8             # attention pipeline depth in (head, key-tile) slots


def _emit(nc, xq, xkv, wq, wk, wv, wo, pb, cmask, outT):
    with tile.TileContext(nc) as tc:
        with (
            tc.tile_pool(name="pers", bufs=1) as pers,
            tc.tile_pool(name="xp", bufs=1) as xp,
            tc.tile_pool(name="wp", bufs=1) as wp,
            tc.tile_pool(name="wo", bufs=1) as wop,
            tc.tile_pool(name="qt", bufs=1) as qtp,
            tc.tile_pool(name="kt", bufs=1) as ktp,
            tc.tile_pool(name="vt", bufs=1) as vtp,
            tc.tile_pool(name="at", bufs=1) as attp,
            tc.tile_pool(name="ex", bufs=LAG + 2) as exp_pool,
            tc.tile_pool(name="rc", bufs=2) as rcp,
            tc.tile_pool(name="ost", bufs=3) as ostp,
            tc.tile_pool(name="ps", bufs=1, space="PSUM") as ps,
        ):
            # ---------------- persistent small tiles ----------------
            pbias_sb = pers.tile([PB, NKT], F32, tag="pbias", name="pbias_sb")
            nc.sync.dma_start(out=pbias_sb[:], in_=pb[:].rearrange("(i p) -> p i", p=PB))


            # ---------------- long-lived activation tiles ----------------
            qt = [qtp.tile([PB, S], F16, tag=f"qt{j}", name=f"qt{j}") for j in range(NJT)]
            kt = [ktp.tile([PB, S], F16, tag=f"kt{j}", name=f"kt{j}") for j in range(NJT)]
            # V with one extra "ones" column per head: (128, 8*65)
            vt = [vtp.tile([PB, HL * (DH + 1)], F16, tag=f"vt{i}", name=f"vt{i}") for i in range(NKT)]
            ones8 = pers.tile([PB, HL], F16, tag="ones8", name="ones8")
            nc.gpsimd.memset(ones8[:], 1.0)
            for i in range(NKT):
                ones_view = vt[i][:].rearrange("p (h c) -> p h c", c=DH + 1)[:, :, DH]
                nc.vector.tensor_copy(ones_view, ones8[:])
            # attention output, transposed layout: att_sb[j][r, q] with
            # r = head-dim row within block j (heads 2j, 2j+1)
            att_sb = [attp.tile([PB, S], F16, tag=f"at{j}", name=f"at{j}") for j in range(NJT)]

            # output-projection weights up front
            wol = []
            for j in range(NJT):
                t = wop.tile([PB, D], F16, tag=f"wo{j}", name=f"wo{j}")
                nc.sync.dma_start(out=t[:], in_=wo[j * PB:(j + 1) * PB, :])
                wol.append(t)

            # PSUM: four (128, 1024) two-bank pair tiles, tags A..D
            def pair_tile(tag):
                return ps.tile([PB, 2 * CH], F32, tag=tag, name=f"ps{tag}")

            def halves(t):
                return [t[:, 0:CH], t[:, CH:2 * CH]]

            def load_w(dram_w, d, pfx="w"):
                t = wp.tile([PB, DG], F16, tag=f"{pfx}{d}", name=f"{pfx}{d}")
                nc.sync.dma_start(out=t[:], in_=dram_w[d * PB:(d + 1) * PB, :])
                return t

            def load_xh(dram_x, d, half):
                t = xp.tile([PB, HS], F16, tag=f"b{half * NDT + d}", name=f"xh{d}")
                nc.sync.dma_start(
                    out=t[:], in_=dram_x[d * PB:(d + 1) * PB,
                                         half * HS:(half + 1) * HS])
                return t

            # ---------------- Q projection (8 groups on A..D) ----------------
            wq_tiles = [load_w(wq, d, "wq") for d in range(NDT)]
            xq_h0 = [load_xh(xq, d, 0) for d in range(NDT)]
            wk_tiles = [load_w(wk, d, "wk") for d in range(NDT)]
            xq_halves = [xq_h0, [load_xh(xq, d, 1) for d in range(NDT)]]
            for half in range(2):
                xh = xq_halves[half]
                accs = []
                for tag in "ABCD":
                    accs += halves(pair_tile(tag))
                for d in range(NDT):
                    for j in range(NJT):
                        for ci in range(2):
                            nc.tensor.matmul(
                                accs[j * 2 + ci],
                                wq_tiles[d][:, j * PB:(j + 1) * PB],
                                xh[d][:, ci * CH:(ci + 1) * CH],
                                start=(d == 0), stop=(d == NDT - 1),
                            )
                for j in range(NJT):
                    for ci in range(2):
                        c = half * 2 + ci
                        nc.vector.tensor_copy(
                            qt[j][:, c * CH:(c + 1) * CH], accs[j * 2 + ci])

            # ---- K projection pass (heads 2jp*2..): one jp, one half, A/B ----
            def k_pass(xh, half, jp):
                accs = halves(pair_tile("A")) + halves(pair_tile("B"))
                for d in range(NDT):
                    for jj in range(2):
                        j = jp * 2 + jj
                        for ci in range(2):
                            nc.tensor.matmul(
                                accs[jj * 2 + ci],
                                wk_tiles[d][:, j * PB:(j + 1) * PB],
                                xh[d][:, ci * CH:(ci + 1) * CH],
                                start=(d == 0), stop=(d == NDT - 1),
                            )
                for jj in range(2):
                    j = jp * 2 + jj
                    for ci in range(2):
                        c = half * 2 + ci
                        nc.vector.tensor_copy(
                            kt[j][:, c * CH:(c + 1) * CH], accs[jj * 2 + ci])

            def v_pass(xh, half, sp, wv_tiles):
                accs = halves(pair_tile("A")) + halves(pair_tile("B"))
                for d in range(NDT):
                    for s4 in range(4):
                        si = sp * 4 + s4
                        nc.tensor.matmul(
                            accs[s4],
                            xh[d][:, si * PB:(si + 1) * PB],
                            wv_tiles[d][:],
                            start=(d == 0), stop=(d == NDT - 1),
                        )
                for s4 in range(4):
                    i = half * 8 + sp * 4 + s4
                    src = accs[s4].rearrange("p (h c) -> p h c", c=DH)
                    dst = vt[i][:].rearrange("p (h c) -> p h c", c=DH + 1)[:, :, 0:DH]
                    nc.vector.tensor_copy(dst, src)

            # kt[0], kt[1] now (heads 0..3); kt[2], kt[3], V deferred into
            # the attention stream as PE filler so exp starts ~100us earlier
            xkv_halves = []
            for half in range(2):
                xh = [load_xh(xkv, d, half) for d in range(NDT)]
                xkv_halves.append(xh)
                k_pass(xh, half, 0)
            wv_tiles = [load_w(wv, d, "wv") for d in range(NDT)]

            # ---------------- attention, lag-LAG pipelined ----------------
            # scores/exp on C/D pair tiles; AV accumulators: chunk c ->
            # half (c%2) of pair (A if c<2 else B), per head.
            state = {"st_cnt": 0, "opair": None}

            def emit_scores(h, i):
                jq = h // 2
                rowo = (h % 2) * DH
                c0 = i // 4
                ex_t = exp_pool.tile([PB, S], F16, tag="ex", name=f"ex_{h}_{i}")
                for hh in range(c0 // 2, 2):
                    st_t = pair_tile("CD"[state["st_cnt"] % 2])
                    state["st_cnt"] += 1
                    lo_c = max(c0, hh * 2)
                    for c in range(lo_c, hh * 2 + 2):
                        q_lo = max(c * CH, i * PB)
                        nc.tensor.matmul(
                            st_t[:, q_lo - hh * 2 * CH:(c - hh * 2 + 1) * CH],
                            kt[jq][rowo:rowo + DH, i * PB:(i + 1) * PB],
                            qt[jq][rowo:rowo + DH, q_lo:(c + 1) * CH],
                            start=True, stop=True,
                        )
                    s0 = max(lo_c * CH, i * PB)
                    span = (hh + 1) * 2 * CH - s0
                    nc.scalar.activation(
                        ex_t[:, s0:s0 + span],
                        st_t[:, s0 - hh * 2 * CH:s0 - hh * 2 * CH + span],
                        mybir.ActivationFunctionType.Exp,
                        bias=pbias_sb[:, i:i + 1], scale=SCALE,
                    )
                # zero q < k inside the 128-wide diagonal block (gpsimd:
                # same firmware lib as partition_broadcast -- no lib thrash)
                nc.gpsimd.affine_select(
                    out=ex_t[:, i * PB:(i + 1) * PB],
                    in_=ex_t[:, i * PB:(i + 1) * PB],
                    compare_op=mybir.AluOpType.is_ge, fill=0.0,
                    base=0, pattern=[[1, PB]],
                    channel_multiplier=-1,
                )
                return ex_t

            def oaug(h, c):
                return state["opair"][c // 2][:, (c % 2) * CH:(c % 2 + 1) * CH]

            def att_rows(h):
                return att_sb[h // 2][(h % 2) * DH:(h % 2) * DH + DH, :]

            def chunk_done(h, c, stg, dnh):
                # free the PSUM bank fast: O rows -> staging at partitions
                # 0..63, denominator row -> dnh at partition 64 (engines
                # cannot shift partitions; DMA cannot read PSUM)
                oa = oaug(h, c)
                nc.vector.tensor_copy(stg[:, c * CH:(c + 1) * CH], oa[0:DH, :])
                nc.vector.tensor_copy(dnh[DH:DH + 1, c * CH:(c + 1) * CH],
                                      oa[DH:DH + 1, :])

            def head_done(h, dnh):
                # reciprocal of all 2048 denominators via the partition-
                # transpose DMA trick (recip free-size 16, not 2048), then
                # broadcast 1/den across partitions. The normalize multiply
                # itself is deferred (emitted a head later) so this chain
                # never blocks the DVE queue.
                dnp = rcp.tile([PB, NKT], F32, tag="dnp", name="dnp_t")
                nc.sync.dma_start(out=dnp[:], in_=dnh[DH:DH + 1, :])
                rcs = rcp.tile([PB, NKT], F32, tag="rcs", name="rcs_t")
                with nc.allow_low_precision(reason="softmax recip"):
                    nc.vector.reciprocal(rcs[:], dnp[:])
                rc2 = rcp.tile([1, S], F32, tag="rc2", bufs=1, name="rc2_t")
                nc.sync.dma_start(out=rc2[:], in_=rcs[:])
                bc = rcp.tile([DH, S], F32, tag="bc", bufs=1, name="bc_t")
                nc.gpsimd.partition_broadcast(bc[:], rc2[0:1, :])
                return bc

            def head_mult(h, stg, bc):
                nc.vector.tensor_tensor(att_rows(h), stg[:], bc[:],
                                        mybir.AluOpType.mult)

            def emit_av(h, i, ex_t):
                if i == 0:
                    # deferred normalize of the PREVIOUS head: its bc has
                    # had a full head of slack to land
                    if state.get("pending") is not None:
                        head_mult(*state["pending"])
                        state["pending"] = None
                    state["opair"] = [pair_tile("A"), pair_tile("B")]
                    state["dnh"] = rcp.tile([DH + 1, S], F32, tag="dnh", bufs=1, name="dnh_t")
                    state["stg"] = rcp.tile([DH, S], F16, tag="stg", name="stg_t")
                c0 = i // 4
                for c in range(c0, NCH):
                    if c == c0:
                        off = i * PB - c0 * CH
                        out_ap = oaug(h, c)[:, off:CH]
                        rhs = ex_t[:, i * PB:(c0 + 1) * CH]
                    else:
                        out_ap = oaug(h, c)
                        rhs = ex_t[:, c * CH:(c + 1) * CH]
                    nc.tensor.matmul(
                        out_ap[0:DH + 1, :],
                        vt[i][:, h * (DH + 1):(h + 1) * (DH + 1)],
                        rhs,
                        start=(i == 0), stop=(i == 4 * c + 3),
                    )
                    if i == 4 * c + 3:
                        chunk_done(h, c, state["stg"], state["dnh"])
                if i == NKT - 1:
                    bc = head_done(h, state["dnh"])
                    state["pending"] = (h, state["stg"], bc)

            # V passes fill the first score slots (PSUM A/B is free until
            # the first AV); K-jp1 passes fill the h1/h2 AV boundaries
            score_fill = {
                0: lambda: v_pass(xkv_halves[0], 0, 0, wv_tiles),
                2: lambda: v_pass(xkv_halves[0], 0, 1, wv_tiles),
                4: lambda: v_pass(xkv_halves[1], 1, 0, wv_tiles),
                6: lambda: v_pass(xkv_halves[1], 1, 1, wv_tiles),
            }
            av_fill = {
                1: lambda: k_pass(xkv_halves[0], 0, 1),
                2: lambda: k_pass(xkv_halves[1], 1, 1),
            }

            def do_av(idx):
                h2, i2 = seq[idx]
                if i2 == 0 and h2 in av_fill:
                    av_fill.pop(h2)()
                emit_av(h2, i2, ex_map.pop((h2, i2)))

            seq = [(h, i) for h in range(HL) for i in range(NKT)]
            ex_map = {}
            for idx, (h, i) in enumerate(seq):
                ex_map[(h, i)] = emit_scores(h, i)
                if idx in score_fill:
                    score_fill.pop(idx)()
                if idx >= LAG:
                    do_av(idx - LAG)
            for idx in range(len(seq) - LAG, len(seq)):
                do_av(idx)
            head_mult(*state["pending"])

            # ---------------- output projection ----------------
            for m in range(D // PB):
                for c in range(NCH):
                    acc = pair_tile("ABCD"[c % 4])[:, 0:CH]
                    for j in range(NJT):
                        nc.tensor.matmul(
                            acc,
                            wol[j][:, m * PB:(m + 1) * PB],
                            att_sb[j][:, c * CH:(c + 1) * CH],
                            start=(j == 0), stop=(j == NJT - 1),
                        )
                    ost = ostp.tile([PB, CH], F16, tag="ost", name="ost")
                    nc.vector.tensor_copy(ost[:], acc)
                    nc.sync.dma_start(
                        out=outT[m * PB:(m + 1) * PB, c * CH:(c + 1) * CH],
                        in_=ost[:])


def build_module():
    nc = bacc.Bacc()
    xq = nc.declare_dram_parameter("xqT", [D, S], F16, isOutput=False)
    xkv = nc.declare_dram_parameter("xkvT", [D, S], F16, isOutput=False)
    wq = nc.declare_dram_parameter("wqT", [D, DG], F16, isOutput=False)
    wk = nc.declare_dram_parameter("wkT", [D, DG], F16, isOutput=False)
    wv = nc.declare_dram_parameter("wvT", [D, DG], F16, isOutput=False)
    wo = nc.declare_dram_parameter("woT", [DG, D], F16, isOutput=False)
    pb = nc.declare_dram_parameter("pbias", [S], F32, isOutput=False)
    cm = nc.declare_dram_parameter("cmask", [2 * PB, PB], F16, isOutput=False)
    outT = nc.declare_dram_parameter("outT", [D, S], F16, isOutput=True)
    _emit(nc, xq, xkv, wq, wk, wv, wo, pb, cm, outT)
    nc.finalize()
    return nc


_NC = None


def _get_nc():
    global _NC
    if _NC is None:
        _NC = build_module()
    return _NC


def make_in_maps(q_raw, kv_raw, padding_mask, Wq, Wk, Wv, Wo):
    q_raw = np.asarray(q_raw, np.float32)
    kv_raw = np.asarray(kv_raw, np.float32)
    qT = np.ascontiguousarray(q_raw.transpose(0, 2, 1)).astype(np.float16)
    kvT = np.ascontiguousarray(kv_raw.transpose(0, 2, 1)).astype(np.float16)
    pbias = np.where(np.asarray(padding_mask) == 0, -1e9, 0.0).astype(np.float32)
    # rows 0..127: mneg = strict-upper -1000 (lhsT of the causal mask add,
    # so that (mneg.T)[k, q] = -1000 where k > q); rows 128..255: identity
    mneg = np.triu(np.full((PB, PB), -1000.0, np.float16), k=1)
    cmask = np.concatenate([mneg, np.eye(PB, dtype=np.float16)], axis=0)
    Wq, Wk, Wv, Wo = (np.asarray(w, np.float32) for w in (Wq, Wk, Wv, Wo))
    wqT = [np.ascontiguousarray(Wq[g * DG:(g + 1) * DG, :].T).astype(np.float16) for g in range(NG)]
    wkT = [np.ascontiguousarray(Wk[g * DG:(g + 1) * DG, :].T).astype(np.float16) for g in range(NG)]
    wvT = [np.ascontiguousarray(Wv[g * DG:(g + 1) * DG, :].T).astype(np.float16) for g in range(NG)]
    woT = [np.ascontiguousarray(Wo[:, g * DG:(g + 1) * DG].T).astype(np.float16) for g in range(NG)]
    in_maps = []
    for c in range(NG * B):
        b, g = divmod(c, NG)
        in_maps.append({
            "xqT": qT[b], "xkvT": kvT[b],
            "wqT": wqT[g], "wkT": wkT[g], "wvT": wvT[g], "woT": woT[g],
            "pbias": pbias[b], "cmask": cmask,
        })
    return in_maps


def kernel(q_raw, kv_raw, padding_mask, Wq, Wk, Wv, Wo):
    from concourse.bass_utils import run_bass_kernel_spmd

    nc = _get_nc()
    in_maps = make_in_maps(q_raw, kv_raw, padding_mask, Wq, Wk, Wv, Wo)
    res = run_bass_kernel_spmd(nc, in_maps, core_ids=list(range(NG * B)))
    out = np.empty((B, S, D), np.float32)
    for b in range(B):
        out[b] = (res.results[NG * b]["outT"].astype(np.float32)
                  + res.results[NG * b + 1]["outT"].astype(np.float32)).T
    return out


# revision 32
# speedup vs baseline: 1.1645x; 1.0820x over previous
"""Distributed causal multi-head attention for 8 TRN2 NeuronCores (v2, fp16).

Problem: B=4, S=2048, D=1024, H=16 heads of DH=64, fp32 in/out, causal mask.

Sharding: core c -> (batch b = c//2, head-group g = c%2 of 8 heads).

v2 changes vs baseline:
- fp16 datapath end-to-end (host pre-casts inputs; matmuls 1 cyc/row vs
  fp32r's 2; halved DMA + SBUF footprint). Verified numerics: ~5e-4 rel.
- attention software-pipelined with lag-3: scores(h,i)+exp emitted 3
  slots ahead of AV(h,i) so the PE never waits on the Scalar-engine exp
  (which is the #2 engine at ~150us total).
- diagonal causal mask via one elementwise multiply with a constant
  triangular tile (gpsimd) instead of affine_select.
- attT kept in SBUF (no DRAM round-trip before the output projection).
- softmax normalize: reciprocal of the denominator row directly (1,512),
  partition_broadcast, one multiply PSUM->SBUF(fp16).
"""

import numpy as np

import concourse.bass as bass
import concourse.mybir as mybir
import concourse.tile as tile
from concourse import bacc

B, S, D, H = 4, 2048, 1024, 16
DH = 64
NG = 2              # head groups (cores per batch)
DG = D // NG        # 512 head dims per core
HL = H // NG        # 8 heads per core
PB = 128            # partition block
CH = 512            # free-dim chunk (one fp32 PSUM bank)
NCH = S // CH       # 4 chunks
NKT = S // PB       # 16 key tiles
NDT = D // PB       # 8 contraction tiles for projections
NJT = DG // PB      # 4 head-dim tiles per core
HS = S // 2         # 1024, half of seq
F32 = mybir.dt.float32
F16 = mybir.dt.float16
SCALE = 1.0 / 8.0   # 1/sqrt(DH)
LAG = 8             # attention pipeline depth in (head, key-tile) slots


def _emit(nc, xq, xkv, wq, wk, wv, wo, pb, cmask, outT):
    with tile.TileContext(nc) as tc:
        with (
            tc.tile_pool(name="pers", bufs=1) as pers,
            tc.tile_pool(name="xp", bufs=1) as xp,
            tc.tile_pool(name="wp", bufs=1) as wp,
            tc.tile_pool(name="wo", bufs=1) as wop,
            tc.tile_pool(name="qt", bufs=1) as qtp,
            tc.tile_pool(name="kt", bufs=1) as ktp,
            tc.tile_pool(name="vt", bufs=1) as vtp,
            tc.tile_pool(name="at", bufs=1) as attp,
            tc.tile_pool(name="ex", bufs=LAG + 2) as exp_pool,
            tc.tile_pool(name="rc", bufs=2) as rcp,
            tc.tile_pool(name="ost", bufs=3) as ostp,
            tc.tile_pool(name="ps", bufs=1, space="PSUM") as ps,
        ):
            # ---------------- persistent small tiles ----------------
            pbias_sb = pers.tile([PB, NKT], F32, tag="pbias", name="pbias_sb")
            nc.sync.dma_start(out=pbias_sb[:], in_=pb[:].rearrange("(i p) -> p i", p=PB))


            # ---------------- long-lived activation tiles ----------------
            qt = [qtp.tile([PB, S], F16, tag=f"qt{j}", name=f"qt{j}") for j in range(NJT)]
            kt = [ktp.tile([PB, S], F16, tag=f"kt{j}", name=f"kt{j}") for j in range(NJT)]
            # V with one extra "ones" column per head: (128, 8*65)
            vt = [vtp.tile([PB, HL * (DH + 1)], F16, tag=f"vt{i}", name=f"vt{i}") for i in range(NKT)]
            ones8 = pers.tile([PB, HL], F16, tag="ones8", name="ones8")
            nc.gpsimd.memset(ones8[:], 1.0)
            for i in range(NKT):
                ones_view = vt[i][:].rearrange("p (h c) -> p h c", c=DH + 1)[:, :, DH]
                nc.vector.tensor_copy(ones_view, ones8[:])
            # attention output, transposed layout: att_sb[j][r, q] with
            # r = head-dim row within block j (heads 2j, 2j+1)
            att_sb = [attp.tile([PB, S], F16, tag=f"at{j}", name=f"at{j}") for j in range(NJT)]

            # output-projection weights up front
            wol = []
            for j in range(NJT):
                t = wop.tile([PB, D], F16, tag=f"wo{j}", name=f"wo{j}")
                nc.sync.dma_start(out=t[:], in_=wo[j * PB:(j + 1) * PB, :])
                wol.append(t)

            # PSUM: four (128, 1024) two-bank pair tiles, tags A..D
            def pair_tile(tag):
                return ps.tile([PB, 2 * CH], F32, tag=tag, name=f"ps{tag}")

            def halves(t):
                return [t[:, 0:CH], t[:, CH:2 * CH]]

            def load_w(dram_w, d, pfx="w"):
                t = wp.tile([PB, DG], F16, tag=f"{pfx}{d}", name=f"{pfx}{d}")
                nc.sync.dma_start(out=t[:], in_=dram_w[d * PB:(d + 1) * PB, :])
                return t

            def load_xh(dram_x, d, half):
                t = xp.tile([PB, HS], F16, tag=f"b{half * NDT + d}", name=f"xh{d}")
                nc.sync.dma_start(
                    out=t[:], in_=dram_x[d * PB:(d + 1) * PB,
                                         half * HS:(half + 1) * HS])
                return t

            # ------------- Q / K-jp0 projections: 2-group passes -------------
            # each pass: one head-dim block j, one seq half, accumulating
            # 8 d-steps into the two halves of a rotating C/D/E pair tile
            # (rotation depth 3 avoids inter-pass PSUM WAR stalls)
            rot = {"n": 0}

            def proj_pass2(w_tiles, out_t, xh, j, half):
                accs = halves(pair_tile("CDE"[rot["n"] % 3]))
                rot["n"] += 1
                for d in range(NDT):
                    for ci in range(2):
                        nc.tensor.matmul(
                            accs[ci],
                            w_tiles[d][:, j * PB:(j + 1) * PB],
                            xh[d][:, ci * CH:(ci + 1) * CH],
                            start=(d == 0), stop=(d == NDT - 1),
                        )
                for ci in range(2):
                    c = half * 2 + ci
                    nc.vector.tensor_copy(
                        out_t[j][:, c * CH:(c + 1) * CH], accs[ci])

            wq_tiles = [load_w(wq, d, "wq") for d in range(NDT)]
            xq_h0 = [load_xh(xq, d, 0) for d in range(NDT)]
            wk_tiles = [load_w(wk, d, "wk") for d in range(NDT)]
            xq_halves = [xq_h0, [load_xh(xq, d, 1) for d in range(NDT)]]
            for half in range(2):
                for j in range(NJT):
                    proj_pass2(wq_tiles, qt, xq_halves[half], j, half)

            # kt[0], kt[1] now (heads 0..3); kt[2], kt[3], V deferred into
            # the attention stream as PE filler so exp starts ~100us earlier
            xkv_halves = []
            for half in range(2):
                xh = [load_xh(xkv, d, half) for d in range(NDT)]
                xkv_halves.append(xh)
                for j in range(2):
                    proj_pass2(wk_tiles, kt, xh, j, half)
            wv_tiles = [load_w(wv, d, "wv") for d in range(NDT)]

            # -------------- attention, chunk-major AV, 3-deep scores --------------
            # scores/exp rotate over three pair tiles C/D/E (rotation depth 3
            # decouples the PE from the Scalar engine's per-act overhead);
            # AV accumulates ONE chunk at a time in a single PSUM bank
            # (VA/VB ping-pong), so ex tiles for the whole head stay live
            # (variable width, bufs=1 -- released before the next head).
            state = {"st_cnt": 0}

            def emit_scores(h, i):
                jq = h // 2
                rowo = (h % 2) * DH
                c0 = i // 4
                # ex_t column 0 = global query column i*PB
                ex_t = exp_pool.tile([PB, S - i * PB], F16, tag=f"ex{i}",
                                     bufs=1, name=f"ex_{h}_{i}")
                for hh in range(c0 // 2, 2):
                    st_t = pair_tile("CDE"[state["st_cnt"] % 3])
                    state["st_cnt"] += 1
                    lo_c = max(c0, hh * 2)
                    for c in range(lo_c, hh * 2 + 2):
                        q_lo = max(c * CH, i * PB)
                        nc.tensor.matmul(
                            st_t[:, q_lo - hh * 2 * CH:(c - hh * 2 + 1) * CH],
                            kt[jq][rowo:rowo + DH, i * PB:(i + 1) * PB],
                            qt[jq][rowo:rowo + DH, q_lo:(c + 1) * CH],
                            start=True, stop=True,
                        )
                    s0 = max(lo_c * CH, i * PB)
                    span = (hh + 1) * 2 * CH - s0
                    nc.scalar.activation(
                        ex_t[:, s0 - i * PB:s0 - i * PB + span],
                        st_t[:, s0 - hh * 2 * CH:s0 - hh * 2 * CH + span],
                        mybir.ActivationFunctionType.Exp,
                        bias=pbias_sb[:, i:i + 1], scale=SCALE,
                    )
                # zero q < k inside the 128-wide diagonal block (gpsimd:
                # same firmware lib as partition_broadcast -- no lib thrash)
                nc.gpsimd.affine_select(
                    out=ex_t[:, 0:PB],
                    in_=ex_t[:, 0:PB],
                    compare_op=mybir.AluOpType.is_ge, fill=0.0,
                    base=0, pattern=[[1, PB]],
                    channel_multiplier=-1,
                )
                return ex_t

            def att_rows(h):
                return att_sb[h // 2][(h % 2) * DH:(h % 2) * DH + DH, :]

            def chunk_done(h, c, acc, stg, dnh):
                # free the PSUM bank fast: O rows -> staging at partitions
                # 0..63, denominator row -> dnh at partition 64 (engines
                # cannot shift partitions; DMA cannot read PSUM)
                nc.vector.tensor_copy(stg[:, c * CH:(c + 1) * CH], acc[0:DH, :])
                nc.vector.tensor_copy(dnh[DH:DH + 1, c * CH:(c + 1) * CH],
                                      acc[DH:DH + 1, :])

            def head_done(h, dnh):
                # reciprocal of all 2048 denominators via the partition-
                # transpose DMA trick (recip free-size 16, not 2048), then
                # broadcast 1/den across partitions. The normalize multiply
                # itself is deferred (emitted a head later) so this chain
                # never blocks the DVE queue.
                dnp = rcp.tile([PB, NKT], F32, tag="dnp", name="dnp_t")
                nc.sync.dma_start(out=dnp[:], in_=dnh[DH:DH + 1, :])
                rcs = rcp.tile([PB, NKT], F32, tag="rcs", name="rcs_t")
                with nc.allow_low_precision(reason="softmax recip"):
                    nc.vector.reciprocal(rcs[:], dnp[:])
                rc2 = rcp.tile([1, S], F32, tag="rc2", bufs=1, name="rc2_t")
                nc.sync.dma_start(out=rc2[:], in_=rcs[:])
                bc = rcp.tile([DH, S], F32, tag="bc", bufs=1, name="bc_t")
                nc.gpsimd.partition_broadcast(bc[:], rc2[0:1, :])
                return bc

            def head_mult(h, stg, bc):
                nc.vector.tensor_tensor(att_rows(h), stg[:], bc[:],
                                        mybir.AluOpType.mult)

            def emit_av_chunk(h, c, ex_map):
                if c == 0:
                    # deferred normalize of the PREVIOUS head: its bc has
                    # had most of a head of slack to land
                    if state.get("pending") is not None:
                        head_mult(*state["pending"])
                        state["pending"] = None
                    state["dnh"] = rcp.tile([DH + 1, S], F32, tag="dnh",
                                            bufs=1, name="dnh_t")
                    state["stg"] = rcp.tile([DH, S], F16, tag="stg", name="stg_t")
                acc = ps.tile([PB, CH], F32, tag="AB"[c % 2] + "V",
                              name=f"av{c % 2}")
                for i in range(4 * c + 4):
                    q_lo = max(c * CH, i * PB)
                    ex_t = ex_map[(h, i)]
                    nc.tensor.matmul(
                        acc[0:DH + 1, q_lo - c * CH:CH],
                        vt[i][:, h * (DH + 1):(h + 1) * (DH + 1)],
                        ex_t[:, q_lo - i * PB:(c + 1) * CH - i * PB],
                        start=(i == 0), stop=(i == 4 * c + 3),
                    )
                chunk_done(h, c, acc, state["stg"], state["dnh"])
                if c == NCH - 1:
                    bc = head_done(h, state["dnh"])
                    state["pending"] = (h, state["stg"], bc)

            # 2-accumulator projection passes (VA+VB banks) used as PE
            # filler inside the attention stream's inter-chunk windows
            def v_pass2(half, spair):
                accs = [ps.tile([PB, CH], F32, tag=t + "V", name="vp")
                        for t in "AB"]
                xh = xkv_halves[half]
                for d in range(NDT):
                    for s2 in range(2):
                        si = spair * 2 + s2
                        nc.tensor.matmul(
                            accs[s2],
                            xh[d][:, si * PB:(si + 1) * PB],
                            wv_tiles[d][:],
                            start=(d == 0), stop=(d == NDT - 1),
                        )
                for s2 in range(2):
                    i = half * 8 + spair * 2 + s2
                    src = accs[s2].rearrange("p (h c) -> p h c", c=DH)
                    dst = vt[i][:].rearrange("p (h c) -> p h c", c=DH + 1)[:, :, 0:DH]
                    nc.vector.tensor_copy(dst, src)

            def k_pass2(half, j):
                accs = [ps.tile([PB, CH], F32, tag=t + "V", name="kp")
                        for t in "AB"]
                xh = xkv_halves[half]
                for d in range(NDT):
                    for ci in range(2):
                        nc.tensor.matmul(
                            accs[ci],
                            wk_tiles[d][:, j * PB:(j + 1) * PB],
                            xh[d][:, ci * CH:(ci + 1) * CH],
                            start=(d == 0), stop=(d == NDT - 1),
                        )
                for ci in range(2):
                    c = half * 2 + ci
                    nc.vector.tensor_copy(
                        kt[j][:, c * CH:(c + 1) * CH], accs[ci])

            # filler placement: (head, after-slot) -> emission closures.
            # V passes deliver vt[i] before the AV chunk that consumes them
            # (AV chunk c of head 0 lands after slot 4c+3 and reads vt[<=4c+3]).
            fills = {
                (0, 0): [lambda: v_pass2(0, 0)],               # vt0,1
                (0, 1): [lambda: v_pass2(0, 1)],               # vt2,3
                (0, 4): [lambda: v_pass2(0, 2)],               # vt4,5
                (0, 5): [lambda: v_pass2(0, 3)],               # vt6,7
                (0, 8): [lambda: v_pass2(1, 0)],               # vt8,9
                (0, 9): [lambda: v_pass2(1, 1)],               # vt10,11
                (0, 12): [lambda: v_pass2(1, 2)],              # vt12,13
                (0, 13): [lambda: v_pass2(1, 3)],              # vt14,15
                (1, 4): [lambda: k_pass2(0, 2)],               # kt2 half0
                (1, 8): [lambda: k_pass2(1, 2)],               # kt2 half1
                (2, 4): [lambda: k_pass2(0, 3)],               # kt3 half0
                (2, 8): [lambda: k_pass2(1, 3)],               # kt3 half1
            }

            ex_map = {}
            for h in range(HL):
                for i in range(NKT):
                    ex_map[(h, i)] = emit_scores(h, i)
                    for f in fills.pop((h, i), []):
                        f()
                    if i % 4 == 3:
                        emit_av_chunk(h, i // 4, ex_map)
                for i in range(NKT):
                    del ex_map[(h, i)]
            head_mult(*state["pending"])

            # ---------------- output projection ----------------
            for m in range(D // PB):
                for c in range(NCH):
                    acc = pair_tile("CDE"[(m * NCH + c) % 3])[:, 0:CH]
                    for j in range(NJT):
                        nc.tensor.matmul(
                            acc,
                            wol[j][:, m * PB:(m + 1) * PB],
                            att_sb[j][:, c * CH:(c + 1) * CH],
                            start=(j == 0), stop=(j == NJT - 1),
                        )
                    ost = ostp.tile([PB, CH], F16, tag="ost", name="ost")
                    nc.vector.tensor_copy(ost[:], acc)
                    nc.sync.dma_start(
                        out=outT[m * PB:(m + 1) * PB, c * CH:(c + 1) * CH],
                        in_=ost[:])


def build_module():
    nc = bacc.Bacc()
    xq = nc.declare_dram_parameter("xqT", [D, S], F16, isOutput=False)
    xkv = nc.declare_dram_parameter("xkvT", [D, S], F16, isOutput=False)
    wq = nc.declare_dram_parameter("wqT", [D, DG], F16, isOutput=False)
    wk = nc.declare_dram_parameter("wkT", [D, DG], F16, isOutput=False)
    wv = nc.declare_dram_parameter("wvT", [D, DG], F16, isOutput=False)
    wo = nc.declare_dram_parameter("woT", [DG, D], F16, isOutput=False)
    pb = nc.declare_dram_parameter("pbias", [S], F32, isOutput=False)
    cm = nc.declare_dram_parameter("cmask", [2 * PB, PB], F16, isOutput=False)
    outT = nc.declare_dram_parameter("outT", [D, S], F16, isOutput=True)
    _emit(nc, xq, xkv, wq, wk, wv, wo, pb, cm, outT)
    nc.finalize()
    return nc


_NC = None


def _get_nc():
    global _NC
    if _NC is None:
        _NC = build_module()
    return _NC


def make_in_maps(q_raw, kv_raw, padding_mask, Wq, Wk, Wv, Wo):
    q_raw = np.asarray(q_raw, np.float32)
    kv_raw = np.asarray(kv_raw, np.float32)
    qT = np.ascontiguousarray(q_raw.transpose(0, 2, 1)).astype(np.float16)
    kvT = np.ascontiguousarray(kv_raw.transpose(0, 2, 1)).astype(np.float16)
    pbias = np.where(np.asarray(padding_mask) == 0, -1e9, 0.0).astype(np.float32)
    # rows 0..127: mneg = strict-upper -1000 (lhsT of the causal mask add,
    # so that (mneg.T)[k, q] = -1000 where k > q); rows 128..255: identity
    mneg = np.triu(np.full((PB, PB), -1000.0, np.float16), k=1)
    cmask = np.concatenate([mneg, np.eye(PB, dtype=np.float16)], axis=0)
    Wq, Wk, Wv, Wo = (np.asarray(w, np.float32) for w in (Wq, Wk, Wv, Wo))
    wqT = [np.ascontiguousarray(Wq[g * DG:(g + 1) * DG, :].T).astype(np.float16) for g in range(NG)]
    wkT = [np.ascontiguousarray(Wk[g * DG:(g + 1) * DG, :].T).astype(np.float16) for g in range(NG)]
    wvT = [np.ascontiguousarray(Wv[g * DG:(g + 1) * DG, :].T).astype(np.float16) for g in range(NG)]
    woT = [np.ascontiguousarray(Wo[:, g * DG:(g + 1) * DG].T).astype(np.float16) for g in range(NG)]
    in_maps = []
    for c in range(NG * B):
        b, g = divmod(c, NG)
        in_maps.append({
            "xqT": qT[b], "xkvT": kvT[b],
            "wqT": wqT[g], "wkT": wkT[g], "wvT": wvT[g], "woT": woT[g],
            "pbias": pbias[b], "cmask": cmask,
        })
    return in_maps


def kernel(q_raw, kv_raw, padding_mask, Wq, Wk, Wv, Wo):
    from concourse.bass_utils import run_bass_kernel_spmd

    nc = _get_nc()
    in_maps = make_in_maps(q_raw, kv_raw, padding_mask, Wq, Wk, Wv, Wo)
    res = run_bass_kernel_spmd(nc, in_maps, core_ids=list(range(NG * B)))
    out = np.empty((B, S, D), np.float32)
    for b in range(B):
        out[b] = (res.results[NG * b]["outT"].astype(np.float32)
                  + res.results[NG * b + 1]["outT"].astype(np.float32)).T
    return out
